# revision 36
# baseline (speedup 1.0000x reference)
"""Multi-head graph attention layer (GAT) for Trainium2, 8-core data-parallel.

Problem: B=8, N=1024, D_IN=256, D_OUT=64, H=8, LeakyReLU slope 0.2.
Sharding: one batch element per NeuronCore.

Algebra: with x = f1_i + f2_j and exp monotone,
  exp(leaky_relu(x)) = E1s_i * U[j,i] / ...  where the per-i factor cancels
in the softmax.  The accumulated unnormalized weight is
  U[j,i] = m[j,i] * max(E2_j * d_i, E2s_j)
with d = exp(0.8 f1), E2 = exp(f2), E2s = exp(0.2 f2).  Two elementwise
passes per (head, j-tile): a (mult,max) tensor_scalar with two per-partition
scalar pointers, then a mask multiply fused over [2 heads x 2 j-tiles]
(quad).  out^T = [Wh|1]^T @ U gives numerators + the denominator row Z.
The finalize transposes out^T via the DMA XBAR (no PE) and normalizes with
a batched broadcast multiply.

Per-(pair, jtile) engine lanes:
  D : DVE ts (mult,max)
  B : ACT relu(E2*d - E2s) then ACT relu(r + E2s)   (frees DVE)
  g : DVE ts + GPSIMD single mask TT (not in quads)
D/B quads share one DVE mask TT over [128, 2, 2, 1024].
"""

import numpy as np
import ml_dtypes

BF16 = ml_dtypes.bfloat16

B, N, D_IN, D_OUT, H = 8, 1024, 256, 64, 8
NEG_SLOPE = 0.2
P = 128
NJT = N // P
NIT = N // P
NKT = D_IN // P
HF = H * D_OUT                # 512
AUG = D_OUT + 1               # 65
TRW = 80                      # transpose row count (65 padded to %16)
NPAIR = H // 2

# lanes: jt 0-3 D, jt 4 B (all pairs), jt 5 D (pairs 0,1) / B (pairs 2,3),
# jt 6-7 g.  Quads cover jt pairs (0,1), (2,3), (4,5).
LANES = {}
for _p in range(NPAIR):
    for _jt in range(NJT):
        if _jt >= 6:
            LANES[(_p, _jt)] = 'g'
        elif _jt == 4:
            LANES[(_p, _jt)] = 'B'
        else:
            LANES[(_p, _jt)] = 'D'


def _build_program():
    import concourse.bass as bass
    import concourse.bacc as bacc
    import concourse.tile as tile
    from concourse import mybir
    from concourse.masks import make_identity

    f32 = mybir.dt.float32
    f32r = mybir.dt.float32r
    bf16 = mybir.dt.bfloat16
    AF = mybir.ActivationFunctionType
    OP = mybir.AluOpType

    nc = bacc.Bacc("TRN2", target_bir_lowering=False, debug=False,
                   enable_asserts=False, num_devices=8)

    hT = nc.dram_tensor("hT", [D_IN, N], f32r, kind="ExternalInput").ap()
    hTb = nc.dram_tensor("hTb", [D_IN, N], bf16, kind="ExternalInput").ap()
    adjT = nc.dram_tensor("adjT", [N, N], bf16, kind="ExternalInput").ap()
    wrsb = nc.dram_tensor("wrsb", [D_IN, HF], bf16,
                          kind="ExternalInput").ap()
    # w12 columns: [w2 | pad | w1] so f2 rows sit at base partition 0 and
    # f1 rows at base partition 32 (engine start-partition rule)
    W12C = 2 * H + 24
    w12 = nc.dram_tensor("w12", [D_IN, W12C], f32r,
                         kind="ExternalInput").ap()
    out = nc.dram_tensor("out", [N, HF], bf16, kind="ExternalOutput").ap()

    with tile.TileContext(nc) as tc:
        with (
            tc.tile_pool(name="const", bufs=1) as const,
            tc.tile_pool(name="inputs", bufs=1) as inputs,
            tc.tile_pool(name="whp", bufs=1) as whp,
            tc.tile_pool(name="ecol", bufs=1) as ecolp,
            tc.tile_pool(name="ps_ot", bufs=3, space="PSUM") as ps_ot,
            tc.tile_pool(name="ps_tr", bufs=1, space="PSUM") as ps_tr,
            tc.tile_pool(name="bcast", bufs=4) as bcastp,
            tc.tile_pool(name="work", bufs=4) as work,
            tc.tile_pool(name="fin", bufs=3) as fin,
            tc.tile_pool(name="dram", bufs=1, space="DRAM") as dramp,
        ):
            # ---- Phase 0: loads -------------------------------------------
            ident = const.tile([P, P], f32)
            make_identity(nc, ident)

            w12_sb = []
            for kt in range(NKT):
                t = inputs.tile([P, W12C], f32r, tag=f"w12{kt}")
                nc.sync.dma_start(out=t, in_=w12[kt * P:(kt + 1) * P, :])
                w12_sb.append(t)
            ht_sb = []
            for kt in range(NKT):
                t = inputs.tile([P, N], f32r, tag=f"ht{kt}")
                for c in range(2):
                    nc.sync.dma_start(
                        out=t[:, 512 * c:512 * (c + 1)],
                        in_=hT[kt * P:(kt + 1) * P, 512 * c:512 * (c + 1)])
                ht_sb.append(t)
            htb_sb = []
            for kt in range(NKT):
                t = inputs.tile([P, N], bf16, tag=f"htb{kt}")
                nc.sync.dma_start(out=t, in_=hTb[kt * P:(kt + 1) * P, :])
                htb_sb.append(t)
            wrs_sb = []
            for kt in range(NKT):
                t = inputs.tile([P, HF], bf16, tag=f"wrs{kt}")
                nc.sync.dma_start(out=t, in_=wrsb[kt * P:(kt + 1) * P, :])
                wrs_sb.append(t)
            adj_all = inputs.tile([P, NJT, N], bf16, tag="adj_all")
            for jt in range(NJT):
                nc.sync.dma_start(out=adj_all[:, jt, :],
                                  in_=adjT[jt * P:(jt + 1) * P, :])

            # ---- Phase 1: f scores (transposed) ---------------------------
            # fT12 rows 0-7 = f2 per head, rows 32-39 = f1 per head
            fT12 = const.tile([W12C, N], f32)
            for half in range(2):
                sl = slice(half * 512, (half + 1) * 512)
                pf = ps_ot.tile([W12C, 512], f32, tag="ot")
                for kt in range(NKT):
                    nc.tensor.matmul(pf, w12_sb[kt], ht_sb[kt][:, sl],
                                     start=(kt == 0), stop=(kt == NKT - 1))
                nc.scalar.copy(fT12[:, sl], pf)

            # d rows = exp(0.8 f1) -> DRAM for partition broadcasts
            dTt = const.tile([W12C, N], bf16)
            nc.scalar.activation(dTt[32:32 + H, :], fT12[32:32 + H, :],
                                 AF.Exp, scale=1.0 - NEG_SLOPE)
            dT_dram = dramp.tile([H, N], bf16)
            nc.sync.dma_start(out=dT_dram, in_=dTt[32:32 + H, :])

            # per i-tile scalars via exp ROWS + one XBAR transpose:
            # ec_all[:, it, 0:8] = E2, [:, it, 8:16] = E2s,
            # [:, it, 16:24] = -E2s
            # rows 8-15 of fT12 are the zero pad columns -> exp gives 1.0
            erows1 = ecolp.tile([2 * H, N], bf16, tag="erows1")
            erows2 = ecolp.tile([2 * H, N], bf16, tag="erows2")
            nc.scalar.activation(erows1, fT12[0:2 * H, :], AF.Exp,
                                 scale=1.0)
            nc.scalar.activation(erows2, fT12[0:2 * H, :], AF.Exp,
                                 scale=NEG_SLOPE)
            ec_b1 = ecolp.tile([P, NIT, 2 * H], bf16, tag="ec_b1")
            ec_b2 = ecolp.tile([P, NIT, 2 * H], bf16, tag="ec_b2")
            nc.sync.dma_start_transpose(out=ec_b1, in_=erows1)
            nc.sync.dma_start_transpose(out=ec_b2, in_=erows2)
            ec_all = ecolp.tile([P, NIT, 3 * H], f32, tag="ec_all")
            nc.vector.tensor_copy(ec_all[:, :, 0:H], ec_b1[:, :, 0:H])
            nc.vector.tensor_copy(ec_all[:, :, H:2 * H], ec_b2[:, :, 0:H])
            nc.vector.tensor_scalar(ec_all[:, :, 2 * H:3 * H],
                                    ec_all[:, :, H:2 * H], -1.0, None,
                                    op0=OP.mult)
            ecols = [ec_all[:, it, :] for it in range(NIT)]

            # ---- Phase 2: Wh (bf16) with aug ones column ------------------
            whaug = []
            for it in range(NIT):
                ps2 = ps_ot.tile([P, H, D_OUT], f32, tag="ot")
                for kt in range(NKT):
                    lhsT = htb_sb[kt][:, it * P:(it + 1) * P]
                    nc.tensor.matmul(ps2, lhsT, wrs_sb[kt],
                                     start=(kt == 0), stop=(kt == NKT - 1))
                wa = whp.tile([P, H, AUG], bf16, tag=f"whaug{it}")
                nc.gpsimd.memset(wa[:, :, D_OUT], 1.0)
                nc.scalar.copy(wa[:, :, 0:D_OUT], ps2)
                whaug.append(wa)

            out_big = whp.tile([P, NIT, HF], bf16, tag="out_big")

            # ---- Phase 3: per head-pair attention -------------------------
            def _fin_transpose_norm(h0, ots_pair):
                for k in range(2):
                    h = h0 + k
                    ots = ots_pair[k]
                    trs = fin.tile([P, NIT, TRW], bf16, tag="trs", bufs=2)
                    nc.sync.dma_start_transpose(out=trs[:, :, :],
                                                in_=ots[0:TRW, :])
                    rc = fin.tile([P, H, 1], f32, tag="rc")
                    nc.vector.reciprocal(rc[:, 0:4, :], trs[:, 0:4, 64:65])
                    nc.vector.reciprocal(rc[:, 4:8, :], trs[:, 4:8, 64:65])
                    for g in range(2):
                        src = trs[:, g * 4:(g + 1) * 4, 0:D_OUT]
                        rcb = rc[:, g * 4:(g + 1) * 4, :] \
                            .broadcast_to([P, 4, D_OUT])
                        dst = out_big[:, g * 4:(g + 1) * 4,
                                      h * D_OUT:(h + 1) * D_OUT]
                        nc.gpsimd.tensor_tensor(out=dst, in0=src,
                                                in1=rcb, op=OP.mult)

            pending = None
            for p in range(NPAIR):
                h0 = 2 * p
                dbc = bcastp.tile([P, 2, N], bf16, tag="dbc")
                for k in range(2):
                    nc.sync.dma_start(
                        out=dbc[:, k, :],
                        in_=dT_dram[h0 + k:h0 + k + 1, :]
                            .partition_broadcast(P))

                ot = [ps_ot.tile([AUG, N], f32, tag="ot", name=f"ot{k}")
                      for k in range(2)]

                # quads over jt pairs (0,1), (2,3), (4,5)
                for q in range(3):
                    j0 = 2 * q
                    tq = work.tile([P, 2, 2, N], bf16, tag="tq")
                    for jl in range(2):
                        jt = j0 + jl
                        for k in range(2):
                            h = h0 + k
                            if LANES[(p, jt)] == 'D':
                                nc.vector.tensor_scalar(
                                    tq[:, k, jl, :], dbc[:, k, :],
                                    ecols[jt][:, h:h + 1],
                                    ecols[jt][:, H + h:H + h + 1],
                                    op0=OP.mult, op1=OP.max)
                            else:  # 'B'
                                r = work.tile([P, N], bf16, tag="rb")
                                nc.scalar.activation(
                                    r, dbc[:, k, :], AF.Relu,
                                    bias=ecols[jt][:,
                                                   2 * H + h:2 * H + h + 1],
                                    scale=ecols[jt][:, h:h + 1])
                                nc.scalar.activation(
                                    tq[:, k, jl, :], r, AF.Relu,
                                    bias=ecols[jt][:, H + h:H + h + 1],
                                    scale=1.0)
                    umq = work.tile([P, 2, 2, N], bf16, tag="umq")
                    adjq = adj_all[:, j0:j0 + 2, :].unsqueeze(1) \
                        .broadcast_to([P, 2, 2, N])
                    nc.vector.tensor_tensor(out=umq, in0=tq, in1=adjq,
                                            op=OP.mult)
                    for jl in range(2):
                        jt = j0 + jl
                        for k in range(2):
                            lhsT = whaug[jt][:, h0 + k, :]
                            for nh in range(2):
                                nc.tensor.matmul(
                                    ot[k][:, nh * 512:(nh + 1) * 512],
                                    lhsT,
                                    umq[:, k, jl, nh * 512:(nh + 1) * 512],
                                    start=(jt == 0), stop=False)
                # jt 6: DVE ts + head-pair-fused DVE mask TT
                tp6 = work.tile([P, 2, N], bf16, tag="tp6")
                for k in range(2):
                    h = h0 + k
                    nc.vector.tensor_scalar(
                        tp6[:, k, :], dbc[:, k, :],
                        ecols[6][:, h:h + 1],
                        ecols[6][:, H + h:H + h + 1],
                        op0=OP.mult, op1=OP.max)
                um6 = work.tile([P, 2, N], bf16, tag="um6")
                adj6 = adj_all[:, 6, :].unsqueeze(1).broadcast_to([P, 2, N])
                nc.vector.tensor_tensor(out=um6, in0=tp6, in1=adj6,
                                        op=OP.mult)
                for k in range(2):
                    lhsT = whaug[6][:, h0 + k, :]
                    for nh in range(2):
                        nc.tensor.matmul(
                            ot[k][:, nh * 512:(nh + 1) * 512], lhsT,
                            um6[:, k, nh * 512:(nh + 1) * 512],
                            start=False, stop=False)
                # jt 7: DVE ts + head-pair-fused DVE mask TT
                tp7 = work.tile([P, 2, N], bf16, tag="tp6")
                for k in range(2):
                    h = h0 + k
                    nc.vector.tensor_scalar(
                        tp7[:, k, :], dbc[:, k, :],
                        ecols[7][:, h:h + 1],
                        ecols[7][:, H + h:H + h + 1],
                        op0=OP.mult, op1=OP.max)
                um7 = work.tile([P, 2, N], bf16, tag="um6")
                adj7 = adj_all[:, 7, :].unsqueeze(1).broadcast_to([P, 2, N])
                nc.vector.tensor_tensor(out=um7, in0=tp7, in1=adj7,
                                        op=OP.mult)
                for k in range(2):
                    lhsT = whaug[7][:, h0 + k, :]
                    for nh in range(2):
                        nc.tensor.matmul(
                            ot[k][:, nh * 512:(nh + 1) * 512], lhsT,
                            um7[:, k, nh * 512:(nh + 1) * 512],
                            start=False, stop=True)

                # evacuate PSUM right away (bf16 numerators, 80 rows for
                # the XBAR transpose granularity; rows 65..79 are garbage)
                ots_pair = []
                for k in range(2):
                    ots = fin.tile([TRW, N], bf16, tag="ots", bufs=4,
                                   name=f"ots{k}")
                    nc.gpsimd.memset(ots[D_OUT:TRW, :], 0.0)
                    nc.scalar.copy(ots[0:AUG, :], ot[k])
                    ots_pair.append(ots)
                if pending is not None:
                    _fin_transpose_norm(*pending)
                pending = (h0, ots_pair)

            _fin_transpose_norm(*pending)

            for it in range(NIT):
                eng = nc.sync if it % 2 == 0 else nc.scalar
                eng.dma_start(out=out[it * P:(it + 1) * P, :],
                              in_=out_big[:, it, :])

    nc.compile()
    return nc


def _host_prep(h, adj, W, a):
    a1, a2 = a[:, :D_OUT], a[:, D_OUT:]
    w1 = np.einsum("hdf,hf->hd", W, a1).astype(np.float32)
    w2 = np.einsum("hdf,hf->hd", W, a2).astype(np.float32)
    w12 = np.concatenate(
        [w2.T, np.zeros((D_IN, 24), np.float32), w1.T], axis=1)
    wrs = np.ascontiguousarray(W.transpose(1, 0, 2).reshape(D_IN, HF))
    in_maps = []
    for b in range(B):
        hTf = np.ascontiguousarray(h[b].T).astype(np.float32)
        in_maps.append({
            "hT": hTf,
            "hTb": hTf.astype(BF16),
            "adjT": np.ascontiguousarray(adj[b].T).astype(BF16),
            "wrsb": wrs.astype(BF16),
            "w12": w12,
        })
    return in_maps


def kernel(h, adj, W, a):
    from concourse.bass_utils import run_bass_kernel_spmd

    in_maps = _host_prep(np.asarray(h), np.asarray(adj),
                         np.asarray(W), np.asarray(a))
    nc = _build_program()
    res = run_bass_kernel_spmd(nc, in_maps, core_ids=list(range(B)))
    out = np.stack([np.asarray(res.results[b]["out"]) for b in range(B)])
    return out.astype(np.float32)


# revision 37
# speedup vs baseline: 1.0546x; 1.0546x over previous
"""Multi-head graph attention layer (GAT) for Trainium2, 8-core data-parallel.

Problem: B=8, N=1024, D_IN=256, D_OUT=64, H=8, LeakyReLU slope 0.2.
Sharding: one batch element per NeuronCore.

Algebra: with x = f1_i + f2_j and exp monotone,
  exp(leaky_relu(x)) = E1s_i * U[j,i] / ...  where the per-i factor cancels
in the softmax.  The accumulated unnormalized weight is
  U[j,i] = m[j,i] * max(E2_j * d_i, E2s_j)
with d = exp(0.8 f1), E2 = exp(f2), E2s = exp(0.2 f2).  Two elementwise
passes per (head, j-tile): a (mult,max) tensor_scalar with two per-partition
scalar pointers, then a mask multiply fused over [2 heads x 2 j-tiles]
(quad).  out^T = [Wh|1]^T @ U gives numerators + the denominator row Z.
The finalize transposes out^T via the DMA XBAR (no PE) and normalizes with
a batched broadcast multiply.

Per-(pair, jtile) engine lanes:
  D : DVE ts (mult,max)
  B : ACT relu(E2*d - E2s) then ACT relu(r + E2s)   (frees DVE)
  g : DVE ts + GPSIMD single mask TT (not in quads)
D/B quads share one DVE mask TT over [128, 2, 2, 1024].
"""

import numpy as np
import ml_dtypes

BF16 = ml_dtypes.bfloat16

B, N, D_IN, D_OUT, H = 8, 1024, 256, 64, 8
NEG_SLOPE = 0.2
P = 128
NJT = N // P
NIT = N // P
NKT = D_IN // P
HF = H * D_OUT                # 512
AUG = D_OUT + 1               # 65
TRW = 80                      # transpose row count (65 padded to %16)
NPAIR = H // 2

# lanes: jt 0-3 D, jt 4 B (all pairs), jt 5 D (pairs 0,1) / B (pairs 2,3),
# jt 6-7 g.  Quads cover jt pairs (0,1), (2,3), (4,5).
LANES = {}
for _p in range(NPAIR):
    for _jt in range(NJT):
        if _jt >= 6:
            LANES[(_p, _jt)] = 'g'
        elif _jt in (4, 5):
            LANES[(_p, _jt)] = 'B'
        else:
            LANES[(_p, _jt)] = 'D'


def _build_program():
    import concourse.bass as bass
    import concourse.bacc as bacc
    import concourse.tile as tile
    from concourse import mybir
    from concourse.masks import make_identity

    f32 = mybir.dt.float32
    f32r = mybir.dt.float32r
    bf16 = mybir.dt.bfloat16
    AF = mybir.ActivationFunctionType
    OP = mybir.AluOpType

    nc = bacc.Bacc("TRN2", target_bir_lowering=False, debug=False,
                   enable_asserts=False, num_devices=8)

    hT = nc.dram_tensor("hT", [D_IN, N], f32r, kind="ExternalInput").ap()
    hTb = nc.dram_tensor("hTb", [D_IN, N], bf16, kind="ExternalInput").ap()
    adjT = nc.dram_tensor("adjT", [N, N], bf16, kind="ExternalInput").ap()
    wrsb = nc.dram_tensor("wrsb", [D_IN, HF], bf16,
                          kind="ExternalInput").ap()
    # w12 columns: [w2 | pad | w1] so f2 rows sit at base partition 0 and
    # f1 rows at base partition 32 (engine start-partition rule)
    W12C = 2 * H + 24
    w12 = nc.dram_tensor("w12", [D_IN, W12C], f32r,
                         kind="ExternalInput").ap()
    out = nc.dram_tensor("out", [N, HF], bf16, kind="ExternalOutput").ap()

    with tile.TileContext(nc) as tc:
        with (
            tc.tile_pool(name="const", bufs=1) as const,
            tc.tile_pool(name="inputs", bufs=1) as inputs,
            tc.tile_pool(name="whp", bufs=1) as whp,
            tc.tile_pool(name="ecol", bufs=1) as ecolp,
            tc.tile_pool(name="ps_ot", bufs=3, space="PSUM") as ps_ot,
            tc.tile_pool(name="ps_tr", bufs=1, space="PSUM") as ps_tr,
            tc.tile_pool(name="bcast", bufs=4) as bcastp,
            tc.tile_pool(name="work", bufs=4) as work,
            tc.tile_pool(name="fin", bufs=3) as fin,
            tc.tile_pool(name="dram", bufs=1, space="DRAM") as dramp,
        ):
            # ---- Phase 0: loads -------------------------------------------
            ident = const.tile([P, P], f32)
            make_identity(nc, ident)

            w12_sb = []
            for kt in range(NKT):
                t = inputs.tile([P, W12C], f32r, tag=f"w12{kt}")
                nc.sync.dma_start(out=t, in_=w12[kt * P:(kt + 1) * P, :])
                w12_sb.append(t)
            ht_sb = []
            for kt in range(NKT):
                t = inputs.tile([P, N], f32r, tag=f"ht{kt}")
                for c in range(2):
                    nc.sync.dma_start(
                        out=t[:, 512 * c:512 * (c + 1)],
                        in_=hT[kt * P:(kt + 1) * P, 512 * c:512 * (c + 1)])
                ht_sb.append(t)
            htb_sb = []
            for kt in range(NKT):
                t = inputs.tile([P, N], bf16, tag=f"htb{kt}")
                nc.sync.dma_start(out=t, in_=hTb[kt * P:(kt + 1) * P, :])
                htb_sb.append(t)
            wrs_sb = []
            for kt in range(NKT):
                t = inputs.tile([P, HF], bf16, tag=f"wrs{kt}")
                nc.sync.dma_start(out=t, in_=wrsb[kt * P:(kt + 1) * P, :])
                wrs_sb.append(t)
            adj_all = inputs.tile([P, NJT, N], bf16, tag="adj_all")
            for jt in range(NJT):
                nc.sync.dma_start(out=adj_all[:, jt, :],
                                  in_=adjT[jt * P:(jt + 1) * P, :])

            # ---- Phase 1: f scores (transposed) ---------------------------
            # fT12 rows 0-7 = f2 per head, rows 32-39 = f1 per head
            fT12 = const.tile([W12C, N], f32)
            for half in range(2):
                sl = slice(half * 512, (half + 1) * 512)
                pf = ps_ot.tile([W12C, 512], f32, tag="ot")
                for kt in range(NKT):
                    nc.tensor.matmul(pf, w12_sb[kt], ht_sb[kt][:, sl],
                                     start=(kt == 0), stop=(kt == NKT - 1))
                nc.scalar.copy(fT12[:, sl], pf)

            # d rows = exp(0.8 f1) -> DRAM for partition broadcasts
            dTt = const.tile([W12C, N], bf16)
            nc.scalar.activation(dTt[32:32 + H, :], fT12[32:32 + H, :],
                                 AF.Exp, scale=1.0 - NEG_SLOPE)
            dT_dram = dramp.tile([H, N], bf16)
            nc.sync.dma_start(out=dT_dram, in_=dTt[32:32 + H, :])

            # per i-tile scalars via exp ROWS + one XBAR transpose:
            # ec_all[:, it, 0:8] = E2, [:, it, 8:16] = E2s,
            # [:, it, 16:24] = -E2s
            # rows 8-15 of fT12 are the zero pad columns -> exp gives 1.0
            erows1 = ecolp.tile([2 * H, N], bf16, tag="erows1")
            erows2 = ecolp.tile([2 * H, N], bf16, tag="erows2")
            nc.scalar.activation(erows1, fT12[0:2 * H, :], AF.Exp,
                                 scale=1.0)
            nc.scalar.activation(erows2, fT12[0:2 * H, :], AF.Exp,
                                 scale=NEG_SLOPE)
            ec_b1 = ecolp.tile([P, NIT, 2 * H], bf16, tag="ec_b1")
            ec_b2 = ecolp.tile([P, NIT, 2 * H], bf16, tag="ec_b2")
            nc.sync.dma_start_transpose(out=ec_b1, in_=erows1)
            nc.sync.dma_start_transpose(out=ec_b2, in_=erows2)
            ec_all = ecolp.tile([P, NIT, 3 * H], f32, tag="ec_all")
            nc.vector.tensor_copy(ec_all[:, :, 0:H], ec_b1[:, :, 0:H])
            nc.vector.tensor_copy(ec_all[:, :, H:2 * H], ec_b2[:, :, 0:H])
            nc.vector.tensor_scalar(ec_all[:, :, 2 * H:3 * H],
                                    ec_all[:, :, H:2 * H], -1.0, None,
                                    op0=OP.mult)
            ecols = [ec_all[:, it, :] for it in range(NIT)]

            # ---- Phase 2: Wh (bf16) with aug ones column ------------------
            whaug = []
            for it in range(NIT):
                ps2 = ps_ot.tile([P, H, D_OUT], f32, tag="ot")
                for kt in range(NKT):
                    lhsT = htb_sb[kt][:, it * P:(it + 1) * P]
                    nc.tensor.matmul(ps2, lhsT, wrs_sb[kt],
                                     start=(kt == 0), stop=(kt == NKT - 1))
                wa = whp.tile([P, H, AUG], bf16, tag=f"whaug{it}")
                nc.gpsimd.memset(wa[:, :, D_OUT], 1.0)
                nc.scalar.copy(wa[:, :, 0:D_OUT], ps2)
                whaug.append(wa)

            out_big = whp.tile([P, NIT, HF], bf16, tag="out_big")

            # ---- Phase 3: per head-pair attention -------------------------
            def _fin_transpose_norm(h0, ots_pair):
                for k in range(2):
                    h = h0 + k
                    ots = ots_pair[k]
                    trs = fin.tile([P, NIT, TRW], bf16, tag="trs", bufs=2)
                    nc.sync.dma_start_transpose(out=trs[:, :, :],
                                                in_=ots[0:TRW, :])
                    rc = fin.tile([P, H, 1], f32, tag="rc")
                    nc.vector.reciprocal(rc[:, 0:4, :], trs[:, 0:4, 64:65])
                    nc.vector.reciprocal(rc[:, 4:8, :], trs[:, 4:8, 64:65])
                    for g in range(2):
                        src = trs[:, g * 4:(g + 1) * 4, 0:D_OUT]
                        rcb = rc[:, g * 4:(g + 1) * 4, :] \
                            .broadcast_to([P, 4, D_OUT])
                        dst = out_big[:, g * 4:(g + 1) * 4,
                                      h * D_OUT:(h + 1) * D_OUT]
                        nc.gpsimd.tensor_tensor(out=dst, in0=src,
                                                in1=rcb, op=OP.mult)

            pending = None
            for p in range(NPAIR):
                h0 = 2 * p
                dbc = bcastp.tile([P, 2, N], bf16, tag="dbc")
                for k in range(2):
                    nc.sync.dma_start(
                        out=dbc[:, k, :],
                        in_=dT_dram[h0 + k:h0 + k + 1, :]
                            .partition_broadcast(P))

                ot = [ps_ot.tile([AUG, N], f32, tag="ot", name=f"ot{k}")
                      for k in range(2)]

                # quads over jt pairs (0,1), (2,3), (4,5)
                for q in range(3):
                    j0 = 2 * q
                    tq = work.tile([P, 2, 2, N], bf16, tag="tq")
                    for jl in range(2):
                        jt = j0 + jl
                        for k in range(2):
                            h = h0 + k
                            if LANES[(p, jt)] == 'D':
                                nc.vector.tensor_scalar(
                                    tq[:, k, jl, :], dbc[:, k, :],
                                    ecols[jt][:, h:h + 1],
                                    ecols[jt][:, H + h:H + h + 1],
                                    op0=OP.mult, op1=OP.max)
                            else:  # 'B'
                                r = work.tile([P, N], bf16, tag="rb")
                                nc.scalar.activation(
                                    r, dbc[:, k, :], AF.Relu,
                                    bias=ecols[jt][:,
                                                   2 * H + h:2 * H + h + 1],
                                    scale=ecols[jt][:, h:h + 1])
                                nc.scalar.activation(
                                    tq[:, k, jl, :], r, AF.Relu,
                                    bias=ecols[jt][:, H + h:H + h + 1],
                                    scale=1.0)
                    umq = work.tile([P, 2, 2, N], bf16, tag="umq")
                    adjq = adj_all[:, j0:j0 + 2, :].unsqueeze(1) \
                        .broadcast_to([P, 2, 2, N])
                    nc.vector.tensor_tensor(out=umq, in0=tq, in1=adjq,
                                            op=OP.mult)
                    for jl in range(2):
                        jt = j0 + jl
                        for k in range(2):
                            lhsT = whaug[jt][:, h0 + k, :]
                            for nh in range(2):
                                nc.tensor.matmul(
                                    ot[k][:, nh * 512:(nh + 1) * 512],
                                    lhsT,
                                    umq[:, k, jl, nh * 512:(nh + 1) * 512],
                                    start=(jt == 0), stop=False)
                # jt 6: DVE ts + head-pair-fused DVE mask TT
                tp6 = work.tile([P, 2, N], bf16, tag="tp6")
                for k in range(2):
                    h = h0 + k
                    nc.vector.tensor_scalar(
                        tp6[:, k, :], dbc[:, k, :],
                        ecols[6][:, h:h + 1],
                        ecols[6][:, H + h:H + h + 1],
                        op0=OP.mult, op1=OP.max)
                um6 = work.tile([P, 2, N], bf16, tag="um6")
                adj6 = adj_all[:, 6, :].unsqueeze(1).broadcast_to([P, 2, N])
                nc.vector.tensor_tensor(out=um6, in0=tp6, in1=adj6,
                                        op=OP.mult)
                for k in range(2):
                    lhsT = whaug[6][:, h0 + k, :]
                    for nh in range(2):
                        nc.tensor.matmul(
                            ot[k][:, nh * 512:(nh + 1) * 512], lhsT,
                            um6[:, k, nh * 512:(nh + 1) * 512],
                            start=False, stop=False)
                # jt 7: DVE ts + head-pair-fused DVE mask TT
                tp7 = work.tile([P, 2, N], bf16, tag="tp6")
                for k in range(2):
                    h = h0 + k
                    nc.vector.tensor_scalar(
                        tp7[:, k, :], dbc[:, k, :],
                        ecols[7][:, h:h + 1],
                        ecols[7][:, H + h:H + h + 1],
                        op0=OP.mult, op1=OP.max)
                um7 = work.tile([P, 2, N], bf16, tag="um6")
                adj7 = adj_all[:, 7, :].unsqueeze(1).broadcast_to([P, 2, N])
                nc.vector.tensor_tensor(out=um7, in0=tp7, in1=adj7,
                                        op=OP.mult)
                for k in range(2):
                    lhsT = whaug[7][:, h0 + k, :]
                    for nh in range(2):
                        nc.tensor.matmul(
                            ot[k][:, nh * 512:(nh + 1) * 512], lhsT,
                            um7[:, k, nh * 512:(nh + 1) * 512],
                            start=False, stop=True)

                # evacuate PSUM right away (bf16 numerators, 80 rows for
                # the XBAR transpose granularity; rows 65..79 are garbage)
                ots_pair = []
                for k in range(2):
                    ots = fin.tile([TRW, N], bf16, tag="ots", bufs=4,
                                   name=f"ots{k}")
                    nc.gpsimd.memset(ots[D_OUT:TRW, :], 0.0)
                    nc.scalar.copy(ots[0:AUG, :], ot[k])
                    ots_pair.append(ots)
                if pending is not None:
                    _fin_transpose_norm(*pending)
                pending = (h0, ots_pair)

            _fin_transpose_norm(*pending)

            for it in range(NIT):
                eng = nc.sync if it % 2 == 0 else nc.scalar
                eng.dma_start(out=out[it * P:(it + 1) * P, :],
                              in_=out_big[:, it, :])

    nc.compile()
    return nc


def _host_prep(h, adj, W, a):
    a1, a2 = a[:, :D_OUT], a[:, D_OUT:]
    w1 = np.einsum("hdf,hf->hd", W, a1).astype(np.float32)
    w2 = np.einsum("hdf,hf->hd", W, a2).astype(np.float32)
    w12 = np.concatenate(
        [w2.T, np.zeros((D_IN, 24), np.float32), w1.T], axis=1)
    wrs = np.ascontiguousarray(W.transpose(1, 0, 2).reshape(D_IN, HF))
    in_maps = []
    for b in range(B):
        hTf = np.ascontiguousarray(h[b].T).astype(np.float32)
        in_maps.append({
            "hT": hTf,
            "hTb": hTf.astype(BF16),
            "adjT": np.ascontiguousarray(adj[b].T).astype(BF16),
            "wrsb": wrs.astype(BF16),
            "w12": w12,
        })
    return in_maps


def kernel(h, adj, W, a):
    from concourse.bass_utils import run_bass_kernel_spmd

    in_maps = _host_prep(np.asarray(h), np.asarray(adj),
                         np.asarray(W), np.asarray(a))
    nc = _build_program()
    res = run_bass_kernel_spmd(nc, in_maps, core_ids=list(range(B)))
    out = np.stack([np.asarray(res.results[b]["out"]) for b in range(B)])
    return out.astype(np.float32)
